# revision 1
# baseline (speedup 1.0000x reference)
"""Behler-Parrinello symmetry-function fingerprints on 8 Trainium2 NeuronCores.

Layout: data-parallel over atoms (1024 atoms/core), partition = atom,
per-atom N*N neighbor-pair work in the free dimension.

Math restructurings vs the reference:
  - cos_jk = u_j . u_k from unit vectors; d_jk via law of cosines
    (sq = dj^2 + dk^2 - 2 dj dk cos), clamped to [0, Rc] so that the
    (1 + cos(pi d/Rc)) factor vanishes at/beyond the cutoff (mask-free).
  - exp(-eta4 (rj^2+rk^2)) * fc(rj) fc(rk) is separable: folded into
    per-neighbor tables h[j], h[k] together with the element masks.
  - (1 +/- cos)^zeta via repeated squaring (zeta = 1,2,4,16).
  - per-feature fused multiply+reduce (scalar_tensor_tensor accum_out)
    with the 0.125 * 2^(1-zeta) constant baked into the scalar operand.
  - g4_11 upper triangle = 0.5 * (full sum - diagonal); diagonal has
    cos = 1, d_jj = 0 so it reduces to an analytic per-neighbor sum.
"""
import numpy as np

A_TOT = 8192
N_NEI = 24
F = 8
N_CORES = 8
A_CORE = A_TOT // N_CORES      # 1024
P = 128                        # partitions (atoms per tile)
NTILES = A_CORE // P           # 8

_BUILT = {}


def _np_reference(n_diff, n_dist, atom_i_idx, j_elems, eta2, R_s, R_c2,
                  zeta, Lambda, eta4, R_c4, n_atoms, n_nei):
    """Pure-numpy fallback (exact reference semantics), chunked over atoms."""
    dt = np.float32
    m1 = (j_elems == 1).astype(dt)
    m8 = (j_elems == 8).astype(dt)

    def fc(d, R_c):
        return 0.5 * (np.cos(np.pi * d / R_c) + 1.0)

    d = n_dist[:, None]
    out_g2 = []
    for m in (m1, m8):
        sf = np.exp(-eta2 * (d - R_s) ** 2) * fc(d, R_c2) * m[:, None]
        acc = np.zeros((n_atoms, F), dt)
        np.add.at(acc, atom_i_idx, sf)
        out_g2.append(acc)

    diff = n_diff.reshape(n_atoms, n_nei, 3)
    dist = n_dist.reshape(n_atoms, n_nei)
    jm1 = m1.reshape(n_atoms, n_nei)
    jm8 = m8.reshape(n_atoms, n_nei)

    def g4(jm, km, same):
        res = np.zeros((n_atoms, F), dt)
        CH = 256
        for s in range(0, n_atoms, CH):
            e = min(s + CH, n_atoms)
            dj = diff[s:e] * jm[s:e][..., None]
            dk = diff[s:e] * km[s:e][..., None]
            rj = dist[s:e] * jm[s:e]
            rk = dist[s:e] * km[s:e]
            dot = np.einsum('anc,amc->anm', dj, dk)
            rp = rj[:, :, None] * rk[:, None, :]
            valid = rp > 0
            if same:
                valid = valid & np.triu(np.ones((n_nei, n_nei), bool), k=1)
            cos = dot / np.where(valid, rp, 1.0)
            sq = ((dk[:, None, :, :] - dj[:, :, None, :]) ** 2).sum(-1)
            djk = np.sqrt(np.where(sq > 0, sq, 1.0))
            djk = np.where(sq > 0, djk, 0.0)
            valid = valid & (djk < R_c4[0])
            p1 = (cos[..., None] * Lambda + 1.0) ** zeta
            p2 = np.exp(-eta4 * (rj[:, :, None] ** 2
                                 + rk[:, None, :] ** 2)[..., None])
            p3 = (fc(rj[:, :, None, None], R_c4) * fc(rk[:, None, :, None],
                                                      R_c4)
                  * fc(djk[..., None], R_c4))
            term = p1 * p2 * p3 * (2.0 ** (1.0 - zeta)) * valid[..., None]
            res[s:e] = term.sum(axis=(1, 2))
        return res

    return np.concatenate([out_g2[0], out_g2[1],
                           g4(jm1, jm8, False), g4(jm1, jm1, True)], axis=1)


# Engine assignment knobs (tuned against the cost-model timeline sim):
#   n_stt_gp: how many of the 16 fused accumulate ops run on GPSIMD
#   sq_plan:  engine per squaring op in chain order ("a"=ACT, "v"=DVE, "g"=GP)
# NOTE: gpsimd.scalar_tensor_tensor does not compile on this toolchain
# (walrus lower_dve rejects it) -> all fused accumulates stay on DVE and
# GPSIMD gets plain tensor_tensor / tensor_scalar work instead.
PLAN = {
    "n_stt_gp": 0,
    "sq_plan": "aaaaaaaa",
    "cc_add_gp": True,
    "gw8_gp": True,
    "t1_gp": True,
    "gh1_split_gp": False,
}


def _build_nc(eta2, R_s, R_c2, zeta, Lambda, eta4u, R_c4u, ntiles=NTILES,
              loop_reps=None, plan=None):
    """Build the per-core Bass program. All hyper-params baked as constants.

    eta4u/R_c4u are uniform scalars (validated by caller). loop_reps wraps
    the whole body in a timing loop (benchmarking only).
    """
    import contextlib
    import concourse.bass as bass
    import concourse.tile as tile
    from concourse import bacc, mybir

    if plan is None:
        plan = PLAN
    f32 = mybir.dt.float32
    Alu = mybir.AluOpType
    Act = mybir.ActivationFunctionType
    N = N_NEI
    rs_zero = bool(np.all(R_s == 0.0))
    rc2_shared = bool(np.all(R_c2 == R_c2[0]))
    rc2u = float(R_c2[0])
    zi = [int(z) for z in zeta]
    assert all(abs(z - iz) < 1e-6 and iz >= 1 for z, iz in zip(zeta, zi))
    # per-feature constant: 2^(1-zeta)/8 (0.125 from the three 0.5 fc factors)
    sc = [0.125 * (2.0 ** (1.0 - z)) for z in zeta]

    nc = bacc.Bacc("TRN2", target_bir_lowering=False, debug=False)
    d_in = nc.dram_tensor("d", [A_CORE, N], f32, kind="ExternalInput")
    xyz_in = nc.dram_tensor("xyz", [A_CORE, 3 * N], f32, kind="ExternalInput")
    m1_in = nc.dram_tensor("m1", [A_CORE, N], f32, kind="ExternalInput")
    m8_in = nc.dram_tensor("m8", [A_CORE, N], f32, kind="ExternalInput")
    out_dr = nc.dram_tensor("out", [A_CORE, 4 * F], f32, kind="ExternalOutput")

    with tile.TileContext(nc) as tc:
        with (
            tc.tile_pool(name="singles", bufs=1) as singles,
            tc.tile_pool(name="io", bufs=3) as io,
            tc.tile_pool(name="small", bufs=2) as small,
            tc.tile_pool(name="big", bufs=3) as big,
        ):
            half_pi = singles.tile([P, 1], f32)
            nc.vector.memset(half_pi[:], float(np.pi / 2))
            ln_half = singles.tile([P, 1], f32)
            nc.vector.memset(ln_half[:], float(np.log(0.5)))

            def emit_tile(it):
                r0, r1 = it * P, (it + 1) * P
                d_t = io.tile([P, N], f32, tag="d_t")
                u = io.tile([P, 3, N], f32, tag="u")
                m1_t = io.tile([P, N], f32, tag="m1_t")
                m8_t = io.tile([P, N], f32, tag="m8_t")
                nc.sync.dma_start(d_t[:], d_in[r0:r1, :])
                nc.sync.dma_start(u[:], xyz_in[r0:r1, :].rearrange(
                    "p (c n) -> p c n", c=3))
                nc.sync.dma_start(m1_t[:], m1_in[r0:r1, :])
                nc.sync.dma_start(m8_t[:], m8_in[r0:r1, :])

                out_t = io.tile([P, 4 * F], f32, tag="out_t")

                # ---- per-neighbor tables -------------------------------
                dsq = small.tile([P, N], f32, tag="dsq")
                nc.gpsimd.tensor_mul(dsq[:], d_t[:], d_t[:])
                rinv = small.tile([P, N], f32, tag="rinv")
                nc.vector.reciprocal(rinv[:], d_t[:])
                nc.vector.tensor_mul(
                    u[:], u[:],
                    rinv[:].unsqueeze(1).broadcast_to([P, 3, N]))
                dcl = small.tile([P, N], f32, tag="dcl")
                nc.gpsimd.tensor_scalar(dcl[:], d_t[:], 0.0, R_c4u,
                                        Alu.max, Alu.min)
                q24 = small.tile([P, N], f32, tag="q24")
                nc.scalar.activation(q24[:], dcl[:], Act.Sin,
                                     bias=half_pi[:],
                                     scale=float(-np.pi / R_c4u))
                e4t = small.tile([P, N], f32, tag="e4t")
                nc.scalar.activation(e4t[:], dsq[:], Act.Exp,
                                     scale=float(-eta4u))
                base = small.tile([P, N], f32, tag="base")
                nc.vector.scalar_tensor_tensor(base[:], q24[:], 1.0, e4t[:],
                                               op0=Alu.add, op1=Alu.mult)
                h1 = small.tile([P, N], f32, tag="h1")
                nc.vector.tensor_mul(h1[:], base[:], m1_t[:])
                h8 = small.tile([P, N], f32, tag="h8")
                nc.vector.tensor_mul(h8[:], base[:], m8_t[:])
                hsq = small.tile([P, N], f32, tag="hsq")
                nc.gpsimd.tensor_mul(hsq[:], h1[:], h1[:])
                hs = small.tile([P, 1], f32, tag="hs")
                nc.vector.reduce_sum(hs[:], hsq[:],
                                     axis=mybir.AxisListType.X)

                # ---- G2 ------------------------------------------------
                if rc2_shared and abs(rc2u - R_c4u) < 1e-12:
                    q22 = q24
                else:
                    q22 = small.tile([P, N], f32, tag="q22")
                    dc2 = small.tile([P, N], f32, tag="dc2")
                    nc.gpsimd.tensor_scalar(dc2[:], d_t[:], 0.0, rc2u,
                                            Alu.max, Alu.min)
                    nc.scalar.activation(q22[:], dc2[:], Act.Sin,
                                         bias=half_pi[:],
                                         scale=float(-np.pi / rc2u))
                hg1 = small.tile([P, N], f32, tag="hg1")
                nc.vector.scalar_tensor_tensor(hg1[:], q22[:], 1.0, m1_t[:],
                                               op0=Alu.add, op1=Alu.mult)
                hg8 = small.tile([P, N], f32, tag="hg8")
                nc.vector.scalar_tensor_tensor(hg8[:], q22[:], 1.0, m8_t[:],
                                               op0=Alu.add, op1=Alu.mult)
                e2b = small.tile([P, F, N], f32, tag="e2b")
                for f in range(F):
                    if rs_zero:
                        nc.scalar.activation(e2b[:, f, :], dsq[:], Act.Exp,
                                             bias=ln_half[:],
                                             scale=float(-eta2[f]))
                    else:
                        dsh = small.tile([P, N], f32, tag="dsh")
                        nc.gpsimd.tensor_scalar_sub(dsh[:], d_t[:],
                                                    float(R_s[f]))
                        dshs = small.tile([P, N], f32, tag="dshs")
                        nc.scalar.square(dshs[:], dsh[:])
                        nc.scalar.activation(e2b[:, f, :], dshs[:], Act.Exp,
                                             bias=ln_half[:],
                                             scale=float(-eta2[f]))
                g2p = small.tile([P, F, N], f32, tag="g2p")
                nc.vector.tensor_mul(
                    g2p[:], e2b[:],
                    hg1[:].unsqueeze(1).broadcast_to([P, F, N]))
                nc.vector.reduce_sum(out_t[:, 0:F], g2p[:],
                                     axis=mybir.AxisListType.X)
                g2p8 = small.tile([P, F, N], f32, tag="g2p8")
                nc.vector.tensor_mul(
                    g2p8[:], e2b[:],
                    hg8[:].unsqueeze(1).broadcast_to([P, F, N]))
                nc.vector.reduce_sum(out_t[:, F:2 * F], g2p8[:],
                                     axis=mybir.AxisListType.X)

                # ---- G4 pair stage -------------------------------------
                def jb(t):   # value varies with j, broadcast along k
                    return t[:].unsqueeze(2).broadcast_to([P, N, N])

                def kb(t):   # value varies with k, broadcast along j
                    return t[:].unsqueeze(1).broadcast_to([P, N, N])

                def jb2(sl):
                    return sl.unsqueeze(2).broadcast_to([P, N, N])

                def kb2(sl):
                    return sl.unsqueeze(1).broadcast_to([P, N, N])

                CC = big.tile([P, N, N], f32, tag="CC")
                tmp1 = big.tile([P, N, N], f32, tag="tmp1")
                tmp2 = big.tile([P, N, N], f32, tag="tmp2")
                ux, uy, uz = u[:, 0, :], u[:, 1, :], u[:, 2, :]
                cc_add_eng = nc.gpsimd if plan["cc_add_gp"] else nc.vector
                nc.vector.tensor_mul(CC[:], jb2(ux), kb2(ux))
                nc.gpsimd.tensor_mul(tmp1[:], jb2(uy), kb2(uy))
                nc.vector.tensor_mul(tmp2[:], jb2(uz), kb2(uz))
                cc_add_eng.tensor_add(CC[:], CC[:], tmp1[:])
                cc_add_eng.tensor_add(CC[:], CC[:], tmp2[:])

                S = big.tile([P, N, N], f32, tag="S")
                nc.gpsimd.tensor_add(S[:], jb(dsq), kb(dsq))
                RP = big.tile([P, N, N], f32, tag="RP")
                nc.gpsimd.tensor_mul(RP[:], jb(d_t), kb(d_t))
                T1 = big.tile([P, N, N], f32, tag="T1")
                (nc.gpsimd if plan["t1_gp"] else nc.vector).tensor_mul(
                    T1[:], RP[:], CC[:])
                SQ = big.tile([P, N, N], f32, tag="SQ")
                nc.vector.scalar_tensor_tensor(SQ[:], T1[:], -2.0, S[:],
                                               op0=Alu.mult, op1=Alu.add)
                SQc = big.tile([P, N, N], f32, tag="SQc")
                nc.gpsimd.tensor_scalar(SQc[:], SQ[:], 0.0, R_c4u * R_c4u,
                                        Alu.max, Alu.min)
                DJK = big.tile([P, N, N], f32, tag="DJK")
                nc.scalar.sqrt(DJK[:], SQc[:])
                Q4 = big.tile([P, N, N], f32, tag="Q4")
                nc.scalar.activation(Q4[:], DJK[:], Act.Sin,
                                     bias=half_pi[:],
                                     scale=float(-np.pi / R_c4u))
                GH1 = big.tile([P, N, N], f32, tag="GH1")
                if plan["gh1_split_gp"]:
                    A4 = big.tile([P, N, N], f32, tag="A4")
                    nc.gpsimd.tensor_scalar_add(A4[:], Q4[:], 1.0)
                    nc.gpsimd.tensor_mul(GH1[:], A4[:], jb(h1))
                else:
                    nc.vector.scalar_tensor_tensor(GH1[:], Q4[:], 1.0,
                                                   jb(h1), op0=Alu.add,
                                                   op1=Alu.mult)
                GW8 = big.tile([P, N, N], f32, tag="GW8")
                (nc.gpsimd if plan["gw8_gp"] else nc.vector).tensor_mul(
                    GW8[:], GH1[:], kb(h8))
                GW1 = big.tile([P, N, N], f32, tag="GW1")
                nc.gpsimd.tensor_mul(GW1[:], GH1[:], kb(h1))

                # powers (1 +/- CC)^z via squaring chains
                need_p = sorted({zi[f] for f in range(F) if Lambda[f] > 0})
                need_m = sorted({zi[f] for f in range(F) if Lambda[f] < 0})
                pows = {}
                sq_ct = [0]

                def mk_sq(dst, src):
                    c = plan["sq_plan"][sq_ct[0] % len(plan["sq_plan"])]
                    if c == "a":
                        nc.scalar.square(dst[:], src[:])
                    elif c == "g":
                        nc.gpsimd.tensor_mul(dst[:], src[:], src[:])
                    else:
                        nc.vector.tensor_mul(dst[:], src[:], src[:])
                    sq_ct[0] += 1

                for sign, need in (("p", need_p), ("m", need_m)):
                    if not need:
                        continue
                    b1 = big.tile([P, N, N], f32, tag=f"pow{sign}1")
                    if sign == "p":
                        nc.vector.tensor_scalar_add(b1[:], CC[:], 1.0)
                    else:
                        nc.vector.tensor_scalar(b1[:], CC[:], -1.0, 1.0,
                                                Alu.mult, Alu.add)
                    pows[(sign, 1)] = b1
                    maxz = max(need)
                    z = 1
                    while z < maxz:
                        src = pows[(sign, z)]
                        z *= 2
                        dst = big.tile([P, N, N], f32, tag=f"pow{sign}{z}")
                        mk_sq(dst, src)
                        pows[(sign, z)] = dst
                    for z in need:
                        if (sign, z) in pows:
                            continue
                        acc = None
                        bit = 1
                        rem = z
                        while rem:
                            if rem & 1:
                                term = pows[(sign, bit)]
                                if acc is None:
                                    acc = term
                                else:
                                    na = big.tile([P, N, N], f32,
                                                  tag=f"pw{sign}{z}a{bit}")
                                    nc.vector.tensor_mul(na[:], acc[:],
                                                         term[:])
                                    acc = na
                            rem >>= 1
                            bit *= 2
                        pows[(sign, z)] = acc

                # fused per-feature multiply+reduce; split across DVE/GPSIMD.
                # Each engine accumulates into its own tiles to avoid
                # cross-engine false deps on a shared output tile.
                scratch = big.tile([P, N, N], f32, tag="scratch")
                scratch_g = big.tile([P, N, N], f32, tag="scratch_g")
                t11v = small.tile([P, F], f32, tag="t11v")
                n_gp = plan["n_stt_gp"]
                stt_i = [0]

                def acc_stt(Pf, scale, GWv, GWg, accv, accg):
                    # distribute the 16 accumulate ops over DVE and GPSIMD
                    i = stt_i[0] % 16
                    use_gp = ((i + 1) * n_gp) // 16 > (i * n_gp) // 16
                    if use_gp:
                        nc.gpsimd.scalar_tensor_tensor(
                            scratch_g[:], Pf[:], float(scale), GWg[:],
                            op0=Alu.mult, op1=Alu.mult, accum_out=accg)
                    else:
                        nc.vector.scalar_tensor_tensor(
                            scratch[:], Pf[:], float(scale), GWv[:],
                            op0=Alu.mult, op1=Alu.mult, accum_out=accv)
                    stt_i[0] += 1
                    return use_gp

                for f in range(F):
                    sign = "p" if Lambda[f] > 0 else "m"
                    Pf = pows[(sign, zi[f])]
                    acc_stt(Pf, sc[f], GW8, GW8,
                            out_t[:, 2 * F + f:2 * F + f + 1],
                            out_t[:, 2 * F + f:2 * F + f + 1])
                    if Lambda[f] > 0:
                        acc11 = t11v[:, f:f + 1]
                    else:
                        acc11 = out_t[:, 3 * F + f:3 * F + f + 1]
                    acc_stt(Pf, 0.5 * sc[f], GW1, GW1, acc11, acc11)
                # diagonal fix for Lambda=+1 features
                for f in range(F):
                    if Lambda[f] > 0:
                        kap = sc[f] * ((1.0 + Lambda[f]) ** zi[f])
                        nc.vector.scalar_tensor_tensor(
                            out_t[:, 3 * F + f:3 * F + f + 1],
                            hs[:], float(-kap), t11v[:, f:f + 1],
                            op0=Alu.mult, op1=Alu.add)

                nc.sync.dma_start(out_dr[r0:r1, :], out_t[:])

            loop_cm = (tc.For_i(0, loop_reps, 1) if loop_reps
                       else contextlib.nullcontext())
            with loop_cm:
                for it in range(ntiles):
                    emit_tile(it)

    nc.compile()
    return nc


def _get_nc(key_arrays):
    key = tuple(np.asarray(a, np.float64).tobytes() for a in key_arrays)
    if key not in _BUILT:
        eta2, R_s, R_c2, zeta, Lambda, eta4, R_c4 = key_arrays
        _BUILT[key] = _build_nc(eta2, R_s, R_c2, zeta, Lambda,
                                float(eta4[0]), float(R_c4[0]))
    return _BUILT[key]


def kernel(n_diff, n_dist, atom_i_idx, j_elems, eta2, R_s, R_c2,
           zeta, Lambda, eta4, R_c4, n_atoms, n_nei):
    n_diff = np.asarray(n_diff, np.float32)
    n_dist = np.asarray(n_dist, np.float32)
    atom_i_idx = np.asarray(atom_i_idx)
    j_elems = np.asarray(j_elems)
    eta2 = np.asarray(eta2, np.float32)
    R_s = np.asarray(R_s, np.float32)
    R_c2 = np.asarray(R_c2, np.float32)
    zeta = np.asarray(zeta, np.float32)
    Lambda = np.asarray(Lambda, np.float32)
    eta4 = np.asarray(eta4, np.float32)
    R_c4 = np.asarray(R_c4, np.float32)
    n_atoms = int(n_atoms)
    n_nei = int(n_nei)

    zi_ok = bool(np.allclose(zeta, np.round(zeta)) and np.all(zeta >= 1))
    idx_ok = bool(np.array_equal(
        atom_i_idx, np.repeat(np.arange(n_atoms, dtype=atom_i_idx.dtype),
                              n_nei)))
    shapes_ok = (n_atoms == A_TOT and n_nei == N_NEI and len(eta2) == F)
    uniform_ok = bool(np.all(eta4 == eta4[0]) and np.all(R_c4 == R_c4[0])
                      and np.all(R_c2 == R_c2[0]))
    if not (zi_ok and idx_ok and shapes_ok and uniform_ok):
        return _np_reference(n_diff, n_dist, atom_i_idx, j_elems, eta2, R_s,
                             R_c2, zeta, Lambda, eta4, R_c4, n_atoms, n_nei)

    from concourse.bass_utils import run_bass_kernel_spmd

    nc = _get_nc((eta2, R_s, R_c2, zeta, Lambda, eta4, R_c4))

    d = n_dist.reshape(A_TOT, N_NEI)
    xyz = np.ascontiguousarray(
        n_diff.reshape(A_TOT, N_NEI, 3).transpose(0, 2, 1)
    ).reshape(A_TOT, 3 * N_NEI)
    m1 = (j_elems == 1).astype(np.float32).reshape(A_TOT, N_NEI)
    m8 = (j_elems == 8).astype(np.float32).reshape(A_TOT, N_NEI)

    in_maps = []
    for c in range(N_CORES):
        s = c * A_CORE
        e = s + A_CORE
        in_maps.append({
            "d": np.ascontiguousarray(d[s:e]),
            "xyz": np.ascontiguousarray(xyz[s:e]),
            "m1": np.ascontiguousarray(m1[s:e]),
            "m8": np.ascontiguousarray(m8[s:e]),
        })

    res = run_bass_kernel_spmd(nc, in_maps, list(range(N_CORES)))
    return np.concatenate([res.results[c]["out"] for c in range(N_CORES)],
                          axis=0)

